# revision 1
# baseline (speedup 1.0000x reference)
"""CRF Viterbi decode (forward max-plus scan + backpointer backtrace + one-hot)
on 8 Trainium2 NeuronCores, data-parallel over the batch axis.

Host contract: kernel(x[256,1024,128] f32, transitions[128,128] f32,
seq_lens[256] i32) -> one_hot(tags)[256,1024,128] f32, bit-matching the jax
reference (first-index argmax tie-breaking).

Per-core layout (BLOC=32 batches as SBUF partitions 0..31, C=128 classes):
  forward step t: for each batch b, scores = Tt + broadcast(alpha[b,:]) is
  computed by a fused tensor_tensor_reduce (add + running max -> m) on DVE;
  the PE broadcasts alpha rows into PSUM via K=1 ones-matmuls (exact fp32);
  max_index extracts the first-argmax backpointer column; backpointers are
  transposed (PE) into [b, j] rows and streamed to DRAM. Sequence-length
  masking is data-driven via copy_predicated so one SPMD program serves all
  cores; lanes are globally sorted by seq_len so a phase schedule (shrinking
  active-lane count) compacts the work.
"""

import sys

sys.path.insert(0, "/opt/trn_rl_repo")

import numpy as np

B, T, C = 256, 1024, 128
NCORES = 8
BLOC = B // NCORES  # 32

_prog_cache = {}


def _assign_lanes(seq_lens: np.ndarray) -> np.ndarray:
    """Snake-deal batches (sorted by seq_len desc) to cores -> [NCORES, BLOC]."""
    order = np.argsort(-seq_lens, kind="stable")
    lanes = [[] for _ in range(NCORES)]
    for i, b in enumerate(order):
        r, k = divmod(i, NCORES)
        c = k if r % 2 == 0 else NCORES - 1 - k
        lanes[c].append(int(b))
    return np.array(lanes, dtype=np.int64)


def _phase_schedule(seq_lens, lanes, ct, t_len, bloc):
    """Per chunk of CT steps, the max (over cores) active-lane count, merged
    into maximal runs [(chunk_lo, chunk_hi, n), ...]."""
    nch = (t_len + ct - 1) // ct
    L = seq_lens[lanes]  # [NCORES, BLOC] descending per row
    ns = []
    for cidx in range(nch):
        t0 = cidx * ct
        n = int(max((row > t0).sum() for row in L))
        n = max(n, 1)
        # quantize up to a few levels so phases span many chunks (keeps the
        # traced instruction count down; For_i loops the chunks of a phase)
        q = bloc // 4 if bloc >= 4 else 1
        n = min(bloc, ((n + q - 1) // q) * q)
        ns.append(n)
    phases = []
    lo = 0
    for cidx in range(1, nch + 1):
        if cidx == nch or ns[cidx] != ns[lo]:
            phases.append((lo, cidx, ns[lo]))
            lo = cidx
    return phases


def build_program(t_len=T, ct=64, bloc=BLOC, phases=None, gs=8, trn="TRN2",
                  num_devices=NCORES):
    """Build the SPMD bass program. Returns (nc, meta)."""
    import concourse.bass as bass
    import concourse.bacc as bacc
    import concourse.mybir as mybir
    from concourse import tile

    f32 = mybir.dt.float32
    u32 = mybir.dt.uint32
    i8 = mybir.dt.int8
    Alu = mybir.AluOpType
    nch = (t_len + ct - 1) // ct
    assert nch * ct == t_len
    if phases is None:
        phases = [(0, nch, bloc)]

    nc = bacc.Bacc(trn, target_bir_lowering=False, debug=False,
                   num_devices=num_devices)

    TC = t_len * C
    x_d = nc.dram_tensor("x", [bloc, TC], f32, kind="ExternalInput").ap()
    tt_d = nc.dram_tensor("tt", [C, C], f32, kind="ExternalInput").ap()
    esel_d = nc.dram_tensor("esel", [bloc, bloc * C], f32, kind="ExternalInput").ap()
    ident_d = nc.dram_tensor("ident", [C, C], f32, kind="ExternalInput").ap()
    iota_d = nc.dram_tensor("iota", [bloc, C], f32, kind="ExternalInput").ap()
    mask_d = nc.dram_tensor("mask", [bloc, t_len], i8, kind="ExternalInput").ap()
    bp_d = nc.dram_tensor("bp", [bloc, TC], f32)  # internal DRAM scratch
    out_d = nc.dram_tensor("out", [bloc, TC], f32, kind="ExternalOutput").ap()

    # persistent SBUF
    tt_sb = nc.alloc_sbuf_tensor("tt_sb", [C, C], f32).ap()
    esel_sb = nc.alloc_sbuf_tensor("esel_sb", [bloc, bloc * C], f32).ap()
    ident_sb = nc.alloc_sbuf_tensor("ident_sb", [C, C], f32).ap()
    iota_sb = nc.alloc_sbuf_tensor("iota_sb", [bloc, C], f32).ap()
    alpha = nc.alloc_sbuf_tensor("alpha", [bloc, C], f32).ap()
    m_t = nc.alloc_sbuf_tensor("m_t", [C, bloc], f32)
    bpu_t = nc.alloc_sbuf_tensor("bpu_t", [C, 8 * bloc], u32)
    bpf = nc.alloc_sbuf_tensor("bpf", [C, bloc], f32).ap()
    tagv = nc.alloc_sbuf_tensor("tagv", [bloc, t_len], f32).ap()
    xr = [nc.alloc_sbuf_tensor(f"xr{p}", [bloc, ct * C], f32).ap() for p in (0, 1)]
    br = [nc.alloc_sbuf_tensor(f"br{p}", [bloc, ct * C], f32).ap() for p in (0, 1)]
    mr = [nc.alloc_sbuf_tensor(f"mr{p}", [bloc, ct], i8).ap() for p in (0, 1)]

    m_ap = m_t.ap()
    bpu_ap = bpu_t.ap()

    def bcast(ap, dim, n):
        """Insert a step-0 (broadcast) dim of size n at position dim."""
        a = ap[tuple(slice(None) for _ in ap.shape)]
        a.ap.insert(dim, [0, n])
        return a

    def m8(b):
        # m_t[:, b] broadcast to free size 8 (for max_index's in_max)
        return bass.AP(m_t, b, [[bloc, C], [0, 8]])

    def bpu_col0():
        # column 0 of each 8-wide slot: [C, bloc] u32 view
        return bass.AP(bpu_t, 0, [[8 * bloc, C], [8, bloc]])

    def mask_col(ring, s, n):
        # mask ring column s broadcast along free C -> [n, C]
        return bass.AP(ring.tensor, ring.offset + s, [[ct, n], [0, C]])

    with tile.TileContext(nc) as tc:
        with (
            tc.tile_pool(name="psA", bufs=2, space="PSUM") as psA,
            tc.tile_pool(name="psS", bufs=2, space="PSUM") as psS,
            tc.tile_pool(name="sbA", bufs=2) as sbA,
            tc.tile_pool(name="sc", bufs=4) as scp,
        ):
            # one-time loads
            nc.sync.dma_start(out=tt_sb, in_=tt_d)
            nc.sync.dma_start(out=esel_sb, in_=esel_d)
            nc.sync.dma_start(out=ident_sb, in_=ident_d)
            nc.sync.dma_start(out=iota_sb, in_=iota_d)
            nc.sync.dma_start(out=alpha, in_=x_d[:, 0:C])  # alpha0 = x[:,0,:]

            def chunk_body(iv, p, n, first_skip):
                """Forward chunk: steps s=0..ct-1 of chunk iv (parity p).
                n = active lanes. first_skip: skip s==0 (t==0) in chunk 0."""
                nc.sync.dma_start(out=xr[p], in_=x_d[:, bass.ds(iv * ct * C, ct * C)])
                nc.sync.dma_start(out=mr[p], in_=mask_d[:, bass.ds(iv * ct, ct)])
                # identity prefill of bp ring (masked steps keep iota rows)
                nc.scalar.copy(out=br[p].rearrange("b (s c) -> b s c", c=C),
                               in_=bcast(iota_sb, 1, ct))
                ng = (n + gs - 1) // gs
                for s in range(ct):
                    if first_skip and s == 0:
                        continue
                    for g in range(ng):
                        b0, b1 = g * gs, min(n, (g + 1) * gs)
                        nb = b1 - b0
                        abuf = psA.tile([C, gs * C], f32, tag="abuf")
                        for k in range(nb):
                            b = b0 + k
                            nc.tensor.matmul(
                                abuf[:, k * C:(k + 1) * C],
                                esel_sb[:, b * C:(b + 1) * C], alpha,
                                start=True, stop=True,
                            )
                        asb = sbA.tile([C, gs * C], f32, tag="asb")
                        nc.scalar.copy(out=asb[:, 0:nb * C], in_=abuf[:, 0:nb * C])
                        sc = scp.tile([C, gs * C], f32, tag="sc")
                        nc.vector.tensor_tensor(
                            out=sc[:, 0:nb * C].rearrange("j (k c) -> j k c", c=C),
                            in0=bcast(tt_sb, 1, nb),
                            in1=asb[:, 0:nb * C].rearrange("j (k c) -> j k c", c=C),
                            op=Alu.add)
                        nc.vector.tensor_reduce(
                            out=m_ap[:, b0:b1],
                            in_=sc[:, 0:nb * C].rearrange("j (k c) -> j k c", c=C),
                            axis=mybir.AxisListType.X, op=Alu.max)
                        for k in range(nb):
                            b = b0 + k
                            nc.vector.max_index(
                                out=bpu_ap[:, b * 8:(b + 1) * 8],
                                in_max=m8(b), in_values=sc[:, k * C:(k + 1) * C],
                            )
                    # bp column extract + transposes + state update
                    nc.vector.tensor_copy(out=bpf[:, 0:n], in_=bass.AP(bpu_t, 0, [[8 * bloc, C], [8, n]]))
                    tpA = psS.tile([bloc, C], f32, tag="tpA")
                    nc.tensor.transpose(tpA[:], m_ap, ident_sb)
                    tpB = psS.tile([bloc, C], f32, tag="tpB")
                    nc.tensor.transpose(tpB[:], bpf, ident_sb)
                    sc2 = scp.tile([bloc, C], f32, tag="sc2")
                    nc.vector.tensor_tensor(
                        out=sc2[0:n, :], in0=tpA[0:n, :],
                        in1=xr[p][0:n, s * C:(s + 1) * C], op=Alu.add)
                    nc.vector.copy_predicated(
                        out=alpha[0:n, :], mask=mask_col(mr[p], s, n),
                        data=sc2[0:n, :])
                    nc.vector.copy_predicated(
                        out=br[p][0:n, s * C:(s + 1) * C],
                        mask=mask_col(mr[p], s, n), data=tpB[0:n, :])
                nc.sync.dma_start(out=bp_d.ap()[:, bass.ds(iv * ct * C, ct * C)],
                                  in_=br[p])

            # ---- forward phases ----
            for (lo, hi, n) in phases:
                def mk(nn, is_first):
                    def body2(iv0, unroll):
                        for u in range(unroll):
                            chunk_body(iv0 + u, u % 2, nn,
                                       first_skip=(is_first and u == 0))
                    return body2
                if lo == 0:
                    # chunk 0 traced alone (skips t=0)
                    tc.For_i_unrolled_general(
                        start=0, end=1, step=1,
                        unrollable_body=mk(n, True), max_unroll=1)
                    if hi > 1:
                        tc.For_i_unrolled_general(
                            start=1, end=hi, step=1,
                            unrollable_body=mk(n, False), max_unroll=1)
                else:
                    tc.For_i_unrolled_general(
                        start=lo, end=hi, step=1,
                        unrollable_body=mk(n, False), max_unroll=1)

            # ---- last tag ----
            mx = scp.tile([bloc, 1], f32, tag="mx")
            nc.vector.tensor_reduce(out=mx[:], in_=alpha, axis=mybir.AxisListType.X,
                                    op=Alu.max)
            lt8 = scp.tile([bloc, 8], u32, tag="lt8")
            mx8 = bass.AP(mx.tensor, mx.offset, [[mx.ap.to_list()[0][0], bloc], [0, 8]])
            nc.vector.max_index(out=lt8[:], in_max=mx8, in_values=alpha)
            nc.vector.tensor_copy(out=tagv[:, t_len - 1:t_len], in_=lt8[:, 0:1])

            # ---- backtrace + one-hot (static chunk loop, reversed) ----
            for cc in range(nch - 1, -1, -1):
                p = cc % 2
                nc.sync.dma_start(out=xr[p], in_=bp_d.ap()[:, cc * ct * C:(cc + 1) * ct * C])
                for s in range(ct - 1, -1, -1):
                    u = cc * ct + s
                    if u == 0:
                        continue
                    oh = scp.tile([bloc, C], f32, tag="oh")
                    nc.vector.tensor_scalar(
                        out=oh[:], in0=iota_sb, scalar1=tagv[:, u:u + 1],
                        scalar2=None, op0=Alu.is_equal)
                    dot = scp.tile([bloc, C], f32, tag="dot")
                    nc.vector.scalar_tensor_tensor(
                        out=dot[:], in0=oh[:], scalar=1.0,
                        in1=xr[p][:, s * C:(s + 1) * C],
                        op0=Alu.mult, op1=Alu.mult,
                        accum_out=tagv[:, u - 1:u])
                # one-hot emit for chunk cc (tags for its steps are final)
                ohc = br[p]
                nc.vector.tensor_tensor(
                    out=ohc.rearrange("b (s c) -> b s c", c=C),
                    in0=bcast(iota_sb, 1, ct),
                    in1=bcast(tagv[:, cc * ct:(cc + 1) * ct], 2, C),
                    op=Alu.is_equal)
                nc.sync.dma_start(out=out_d[:, cc * ct * C:(cc + 1) * ct * C],
                                  in_=ohc)

    nc.compile()
    return nc


def _host_inputs(x, transitions, seq_lens, lanes, t_len=T, bloc=BLOC):
    """Build per-core input maps."""
    tt = np.ascontiguousarray(transitions.T).astype(np.float32)
    esel = np.zeros((bloc, bloc * C), np.float32)
    for b in range(bloc):
        esel[b, b * C:(b + 1) * C] = 1.0
    ident = np.eye(C, dtype=np.float32)
    iota = np.tile(np.arange(C, dtype=np.float32), (bloc, 1))
    in_maps = []
    for c in range(lanes.shape[0]):
        lx = x[lanes[c]][:, :t_len, :].reshape(bloc, t_len * C).astype(np.float32)
        L = seq_lens[lanes[c]].astype(np.int64)
        tgrid = np.arange(t_len)[None, :]
        mask = (tgrid < L[:, None]).astype(np.int8)  # active at step t: t < L
        in_maps.append({
            "x": np.ascontiguousarray(lx),
            "tt": tt, "esel": esel, "ident": ident, "iota": iota,
            "mask": np.ascontiguousarray(mask),
        })
    return in_maps


TRACE = False
LAST_RESULT = None


def kernel(x, transitions, seq_lens):
    global LAST_RESULT
    from concourse.bass_utils import run_bass_kernel_spmd

    x = np.asarray(x, dtype=np.float32)
    transitions = np.asarray(transitions, dtype=np.float32)
    seq_lens = np.asarray(seq_lens)
    lanes = _assign_lanes(seq_lens)
    ct = 64
    phases = _phase_schedule(seq_lens, lanes, ct, T, BLOC)
    key = tuple(phases)
    if key not in _prog_cache:
        _prog_cache[key] = build_program(T, ct, BLOC, phases)
    nc = _prog_cache[key]
    in_maps = _host_inputs(x, transitions, seq_lens, lanes)
    res = run_bass_kernel_spmd(nc, in_maps, list(range(NCORES)), trace=TRACE)
    LAST_RESULT = res
    out = np.empty((B, T, C), np.float32)
    for c in range(NCORES):
        out[lanes[c]] = res.results[c]["out"].reshape(BLOC, T, C)
    return out



# revision 2
# speedup vs baseline: 1.4620x; 1.4620x over previous
"""CRF Viterbi decode — alpha-history forward + recompute backtrace.

Per-core (BLOC=32 batches, C=128 classes, T=1024):

Forward (per step): alpha is decomposed exactly into three bf16 pieces
(alpha = h+m+l, 24 mantissa bits), flattened to [3, 32*128] by SBUF->SBUF
DMAs, and broadcast across all 128 partitions by ONE K=3 ones-matmul per
4-batch block (bf16 moving data at 1 cycle/row; PSUM fp32 accumulation
reconstructs alpha exactly) — replacing the 2x-fp32 per-batch esel matmuls
that made the PE the bottleneck. DVE computes scores = tt + alpha_bcast and
a segmented tensor_reduce gives m[j,k] = max_i scores. No backpointers are
extracted (no per-batch max_index in the forward loop). The candidate alpha
history hist[:, t*32:(t+1)*32] = m + x_t^T stays in SBUF ([128, T*32] f32 =
128KB/partition), unmasked — valid wherever the backtrace reads it because
a lane's active steps are a prefix of [0, T).

Backtrace (per step): recompute only the visited bp entry for all 32 lanes
at once: PSUM B = transpose(hist_col(t-1)) [32,128], then one matmul
accumulates tt[tag_b, :] via a one-hot lhsT, giving score rows with exactly
one fp32 rounding (bit-matching the reference); reduce + max_index give the
predecessor tags with first-index tie-breaking.

One-hot emission per chunk + DMA out (reusing the x ring buffers).
Sequence-length masking via copy_predicated; lanes globally sorted by
seq_len; a phase schedule shrinks the active-lane count over time.
"""

import sys

sys.path.insert(0, "/opt/trn_rl_repo")

import numpy as np

B, T, C = 256, 1024, 128
NCORES = 8
BLOC = B // NCORES  # 32

_prog_cache = {}


def _assign_lanes(seq_lens: np.ndarray) -> np.ndarray:
    """Snake-deal batches (sorted by seq_len desc) to cores -> [NCORES, BLOC]."""
    order = np.argsort(-seq_lens, kind="stable")
    lanes = [[] for _ in range(NCORES)]
    for i, b in enumerate(order):
        r, k = divmod(i, NCORES)
        c = k if r % 2 == 0 else NCORES - 1 - k
        lanes[c].append(int(b))
    return np.array(lanes, dtype=np.int64)


def _phase_schedule(seq_lens, lanes, ct, t_len, bloc, q=4):
    """Per chunk of CT steps, the max (over cores) active-lane count, merged
    into maximal runs [(chunk_lo, chunk_hi, n), ...]."""
    nch = (t_len + ct - 1) // ct
    L = seq_lens[lanes]  # [NCORES, BLOC] descending per row
    ns = []
    for cidx in range(nch):
        t0 = cidx * ct
        n = int(max((row > t0).sum() for row in L))
        n = max(n, 1)
        n = min(bloc, ((n + q - 1) // q) * q)
        ns.append(n)
    phases = []
    lo = 0
    for cidx in range(1, nch + 1):
        if cidx == nch or ns[cidx] != ns[lo]:
            phases.append((lo, cidx, ns[lo]))
            lo = cidx
    return phases


# Tunables
GP_TT = False       # offload part of the scores tensor_tensor to GPSIMD
GP_FRAC = 0.5       # fraction of active batches whose scores go to GPSIMD
GP_EMIT = False     # one-hot emission on GPSIMD (else DVE)


def build_program(t_len=T, ct=32, bloc=BLOC, phases=None, trn="TRN2",
                  num_devices=NCORES, gp_tt=GP_TT, gp_frac=GP_FRAC,
                  gp_emit=GP_EMIT):
    import concourse.bass as bass
    import concourse.bacc as bacc
    import concourse.mybir as mybir
    from concourse import tile

    f32 = mybir.dt.float32
    u32 = mybir.dt.uint32
    i8 = mybir.dt.int8
    Alu = mybir.AluOpType
    nch = (t_len + ct - 1) // ct
    assert nch * ct == t_len
    if phases is None:
        phases = [(0, nch, bloc)]

    nc = bacc.Bacc(trn, target_bir_lowering=False, debug=False,
                   num_devices=num_devices)

    TC = t_len * C
    x_d = nc.dram_tensor("x", [bloc, TC], f32, kind="ExternalInput").ap()
    tt_d = nc.dram_tensor("tt", [C, C], f32, kind="ExternalInput").ap()
    ident_d = nc.dram_tensor("ident", [C, C], f32, kind="ExternalInput").ap()
    iota_d = nc.dram_tensor("iota", [bloc, C], f32, kind="ExternalInput").ap()
    ones3_d = nc.dram_tensor("ones3", [3, C], mybir.dt.bfloat16, kind="ExternalInput").ap()
    mask_d = nc.dram_tensor("mask", [bloc, t_len], i8, kind="ExternalInput").ap()
    out_d = nc.dram_tensor("out", [bloc, TC], f32, kind="ExternalOutput").ap()

    # persistent SBUF
    tt_sb = nc.alloc_sbuf_tensor("tt_sb", [C, C], f32).ap()
    ident_sb = nc.alloc_sbuf_tensor("ident_sb", [C, C], f32).ap()
    iota_sb = nc.alloc_sbuf_tensor("iota_sb", [bloc, C], f32).ap()
    bf16 = mybir.dt.bfloat16
    ones3_sb = nc.alloc_sbuf_tensor("ones3_sb", [3, C], bf16).ap()
    hmlb = nc.alloc_sbuf_tensor("hmlb", [bloc, 3 * C], bf16).ap()
    hmlf = nc.alloc_sbuf_tensor("hmlf", [3, bloc * C], bf16).ap()
    rs1 = nc.alloc_sbuf_tensor("rs1", [bloc, C], f32).ap()
    rs2 = nc.alloc_sbuf_tensor("rs2", [bloc, C], f32).ap()
    alpha = nc.alloc_sbuf_tensor("alpha", [bloc, C], f32).ap()
    hist = nc.alloc_sbuf_tensor("hist", [C, t_len * bloc], f32).ap()
    sc = nc.alloc_sbuf_tensor("sc", [C, bloc * C], f32).ap()   # scores scratch
    m_t = nc.alloc_sbuf_tensor("m_t", [C, bloc], f32)
    tagh = nc.alloc_sbuf_tensor("tagh", [bloc, t_len], f32).ap()
    ohrs = nc.alloc_sbuf_tensor("ohrs", [bloc, C], f32).ap()   # one-hot rows
    ohT = nc.alloc_sbuf_tensor("ohT", [C, bloc], f32).ap()     # one-hot cols
    bpu_t = nc.alloc_sbuf_tensor("bpu_t", [bloc, 8], u32)
    lt8_t = nc.alloc_sbuf_tensor("lt8_t", [bloc, 8], u32)
    mx = nc.alloc_sbuf_tensor("mx", [bloc, 1], f32)
    bpf = nc.alloc_sbuf_tensor("bpf", [bloc, 1], f32).ap()
    bsv = nc.alloc_sbuf_tensor("bsv", [bloc, C], f32).ap()  # fwd cand scratch
    bsc = nc.alloc_sbuf_tensor("bsc", [bloc, C], f32).ap()  # backtrace scores
    xr = [nc.alloc_sbuf_tensor(f"xr{p}", [bloc, ct * C], f32).ap() for p in (0, 1)]
    mr = [nc.alloc_sbuf_tensor(f"mr{p}", [bloc, ct], i8).ap() for p in (0, 1)]

    m_ap = m_t.ap()
    bpu_ap = bpu_t.ap()
    lt8_ap = lt8_t.ap()
    mx_ap = mx.ap()

    if gp_tt:
        ngp = int(round(gp_frac * bloc / 4)) * 4
    else:
        ngp = 0

    def bcast(ap, dim, n):
        """Insert a step-0 (broadcast) dim of size n at position dim."""
        a = ap[tuple(slice(None) for _ in ap.shape)]
        a.ap.insert(dim, [0, n])
        return a

    def mx8():
        return bass.AP(mx, 0, [[1, bloc], [0, 8]])

    def mask_col(ring, s, n):
        return bass.AP(ring.tensor, ring.offset + s, [[ct, n], [0, C]])

    def mask_col1(ring, s, n):
        return bass.AP(ring.tensor, ring.offset + s, [[ct, n], [0, 1]])

    with tile.TileContext(nc) as tc:
        with (
            tc.tile_pool(name="psA", bufs=1, space="PSUM") as psA,
            tc.tile_pool(name="psS", bufs=2, space="PSUM") as psS,
        ):
            # one-time loads
            nc.sync.dma_start(out=tt_sb, in_=tt_d)
            nc.sync.dma_start(out=ident_sb, in_=ident_d)
            nc.sync.dma_start(out=iota_sb, in_=iota_d)
            nc.sync.dma_start(out=ones3_sb, in_=ones3_d)
            nc.sync.dma_start(out=alpha, in_=x_d[:, 0:C])  # alpha0 = x[:,0,:]

            def split_flatten():
                """hml decomposition of alpha (exact: alpha = h+m+l in fp32)
                + flatten to [3, bloc*C] for the broadcast matmul. All on DVE
                so the chain has no cross-engine hops; each piece's flatten
                DMA is issued as soon as that piece exists."""
                nc.vector.tensor_copy(out=hmlb[:, 0:C], in_=alpha)     # h
                nc.sync.dma_start(out=hmlf[0:1, :], in_=hmlb[:, 0:C])
                nc.vector.tensor_tensor(out=rs1, in0=alpha,
                                        in1=hmlb[:, 0:C], op=Alu.subtract)
                nc.vector.tensor_copy(out=hmlb[:, C:2 * C], in_=rs1)   # m
                nc.sync.dma_start(out=hmlf[1:2, :], in_=hmlb[:, C:2 * C])
                nc.vector.tensor_tensor(out=rs2, in0=rs1,
                                        in1=hmlb[:, C:2 * C], op=Alu.subtract)
                nc.vector.tensor_copy(out=hmlb[:, 2 * C:3 * C], in_=rs2)  # l
                nc.sync.dma_start(out=hmlf[2:3, :], in_=hmlb[:, 2 * C:3 * C])

            # hist col 0 = alpha0^T (PE transpose into PSUM, copy to SBUF)
            tp0 = psS.tile([C, bloc], f32, tag="tpX")
            nc.tensor.transpose(tp0[:], alpha, ident_sb[0:bloc, 0:bloc])
            nc.scalar.copy(out=hist[:, 0:bloc], in_=tp0[:])
            split_flatten()

            def fwd_step(iv, p, s, n):
                """One forward step t = iv*ct + s with n active lanes."""
                nd = max(4, n - ngp) if gp_tt else n  # DVE batches [0, nd)
                # PE: broadcast alpha rows into PSUM, rounds of 8 batches
                abufs = []
                r0 = 0
                while r0 < n:
                    rn = min(8, n - r0)
                    abuf = psA.tile([C, 8 * C], f32, tag=f"ab{(r0 // 8) % 2}")
                    for h0 in range(0, rn, 4):
                        hn = min(4, rn - h0)
                        nc.tensor.matmul(
                            abuf[:, h0 * C:(h0 + hn) * C], ones3_sb,
                            hmlf[:, (r0 + h0) * C:(r0 + h0 + hn) * C],
                            start=True, stop=True,
                        )
                    abufs.append((r0, rn, abuf))
                    r0 += rn
                # scores = tt + abuf  (DVE batches < nd, GPSIMD the rest)
                for (r0, rn, abuf) in abufs:
                    dsp = min(max(nd - r0, 0), rn)
                    if dsp > 0:
                        nc.vector.tensor_tensor(
                            out=sc[:, r0 * C:(r0 + dsp) * C].rearrange(
                                "j (k c) -> j k c", c=C),
                            in0=bcast(tt_sb, 1, dsp),
                            in1=abuf[:, 0:dsp * C].rearrange(
                                "j (k c) -> j k c", c=C),
                            op=Alu.add)
                    if dsp < rn:
                        ng = rn - dsp
                        nc.gpsimd.tensor_tensor(
                            out=sc[:, (r0 + dsp) * C:(r0 + rn) * C].rearrange(
                                "j (k c) -> j k c", c=C),
                            in0=bcast(tt_sb, 1, ng),
                            in1=abuf[:, dsp * C:rn * C].rearrange(
                                "j (k c) -> j k c", c=C),
                            op=Alu.add)
                # m[j, k] = max_i scores
                nc.vector.tensor_reduce(
                    out=m_ap[:, 0:n],
                    in_=sc[:, 0:n * C].rearrange("j (k c) -> j k c", c=C),
                    axis=mybir.AxisListType.X, op=Alu.max)
                # mT and xT via PE transposes
                tpA = psS.tile([bloc, C], f32, tag="tpA")
                nc.tensor.transpose(tpA[:], m_ap, ident_sb)
                tpX = psS.tile([C, bloc], f32, tag="tpX")
                nc.tensor.transpose(
                    tpX[:], xr[p][:, s * C:(s + 1) * C],
                    ident_sb[0:bloc, 0:bloc])
                # hist col t = m + xT (unmasked candidate alpha)
                nc.vector.tensor_tensor(
                    out=hist[:, bass.ds((iv * ct + s) * bloc, bloc)],
                    in0=m_ap[:], in1=tpX[:], op=Alu.add)
                # cand = mT + x_t ; alpha = where(mask, cand, alpha)
                nc.vector.tensor_tensor(
                    out=bsv[0:n, :], in0=tpA[0:n, :],
                    in1=xr[p][0:n, s * C:(s + 1) * C], op=Alu.add)
                nc.vector.copy_predicated(
                    out=alpha[0:n, :], mask=mask_col(mr[p], s, n),
                    data=bsv[0:n, :])
                split_flatten()

            def chunk_body(iv, p, n, first_skip):
                nc.sync.dma_start(out=xr[p], in_=x_d[:, bass.ds(iv * ct * C, ct * C)])
                nc.sync.dma_start(out=mr[p], in_=mask_d[:, bass.ds(iv * ct, ct)])
                for s in range(ct):
                    if first_skip and s == 0:
                        continue
                    fwd_step(iv, p, s, n)

            for (lo, hi, n) in phases:
                def mk(nn, is_first):
                    def body2(iv0, unroll):
                        for u in range(unroll):
                            chunk_body(iv0 + u, u % 2, nn,
                                       first_skip=(is_first and u == 0))
                    return body2
                if lo == 0:
                    tc.For_i_unrolled_general(
                        start=0, end=1, step=1,
                        unrollable_body=mk(n, True), max_unroll=1)
                    if hi > 1:
                        tc.For_i_unrolled_general(
                            start=1, end=hi, step=1,
                            unrollable_body=mk(n, False), max_unroll=1)
                else:
                    tc.For_i_unrolled_general(
                        start=lo, end=hi, step=1,
                        unrollable_body=mk(n, False), max_unroll=1)

            # ---- last tag ----
            nc.vector.tensor_reduce(out=mx_ap[:], in_=alpha,
                                    axis=mybir.AxisListType.X, op=Alu.max)
            nc.vector.max_index(out=lt8_ap, in_max=mx8(), in_values=alpha)
            nc.vector.tensor_copy(out=tagh[:, t_len - 1:t_len],
                                  in_=lt8_ap[:, 0:1])

            # ---- backtrace (static reverse loop) + one-hot emission ----
            for cc in range(nch - 1, -1, -1):
                p = cc % 2
                nc.sync.dma_start(out=mr[p], in_=mask_d[:, cc * ct:(cc + 1) * ct])
                for s in range(ct - 1, -1, -1):
                    t = cc * ct + s
                    if t == 0:
                        continue
                    # one-hot rows of tag_t, PE-transposed to columns
                    nc.vector.tensor_scalar(
                        out=ohrs, in0=iota_sb, scalar1=tagh[:, t:t + 1],
                        scalar2=None, op0=Alu.is_equal)
                    ohTp = psS.tile([C, bloc], f32, tag="tpX")
                    nc.tensor.transpose(ohTp[:], ohrs, ident_sb[0:bloc, 0:bloc])
                    nc.scalar.copy(out=ohT, in_=ohTp[:])
                    # B = alphaF(t-1) rows + tt[tag, :]
                    Bp = psS.tile([bloc, C], f32, tag="tpA")
                    nc.tensor.transpose(
                        Bp[:], hist[:, (t - 1) * bloc:t * bloc], ident_sb)
                    nc.tensor.matmul(Bp[:], ohT, tt_sb, start=False, stop=True,
                                     skip_group_check=True)
                    # argmax over i (first-index)
                    nc.vector.tensor_reduce(
                        out=mx_ap[:], in_=Bp[:], axis=mybir.AxisListType.X,
                        op=Alu.max)
                    nc.scalar.copy(out=bsc, in_=Bp[:])
                    nc.vector.max_index(out=bpu_ap, in_max=mx8(), in_values=bsc)
                    nc.vector.tensor_copy(out=bpf, in_=bpu_ap[:, 0:1])
                    # tagh[t-1] = mask_t ? bp : tagh[t]
                    nc.vector.tensor_copy(out=tagh[:, t - 1:t],
                                          in_=tagh[:, t:t + 1])
                    nc.vector.copy_predicated(
                        out=tagh[:, t - 1:t], mask=mask_col1(mr[p], s, bloc),
                        data=bpf)
                # emit one-hot for chunk cc into the x ring buffer
                ohc = xr[p]
                emit_eng = nc.gpsimd if gp_emit else nc.vector
                emit_eng.tensor_tensor(
                    out=ohc.rearrange("b (s c) -> b s c", c=C),
                    in0=bcast(iota_sb, 1, ct),
                    in1=bcast(tagh[:, cc * ct:(cc + 1) * ct], 2, C),
                    op=Alu.is_equal)
                nc.sync.dma_start(out=out_d[:, cc * ct * C:(cc + 1) * ct * C],
                                  in_=ohc)

    nc.compile()
    return nc


def _host_inputs(x, transitions, seq_lens, lanes, t_len=T, bloc=BLOC):
    tt = np.ascontiguousarray(transitions.T).astype(np.float32)
    ident = np.eye(C, dtype=np.float32)
    iota = np.tile(np.arange(C, dtype=np.float32), (bloc, 1))
    import ml_dtypes
    ones3 = np.ones((3, C), dtype=ml_dtypes.bfloat16)
    in_maps = []
    for c in range(lanes.shape[0]):
        lx = x[lanes[c]][:, :t_len, :].reshape(bloc, t_len * C).astype(np.float32)
        L = seq_lens[lanes[c]].astype(np.int64)
        tgrid = np.arange(t_len)[None, :]
        mask = (tgrid < L[:, None]).astype(np.int8)
        in_maps.append({
            "x": np.ascontiguousarray(lx),
            "tt": tt, "ident": ident, "iota": iota, "ones3": ones3,
            "mask": np.ascontiguousarray(mask),
        })
    return in_maps


TRACE = False
LAST_RESULT = None


def kernel(x, transitions, seq_lens):
    global LAST_RESULT
    from concourse.bass_utils import run_bass_kernel_spmd

    x = np.asarray(x, dtype=np.float32)
    transitions = np.asarray(transitions, dtype=np.float32)
    seq_lens = np.asarray(seq_lens)
    lanes = _assign_lanes(seq_lens)
    ct = 32
    phases = _phase_schedule(seq_lens, lanes, ct, T, BLOC)
    key = tuple(phases)
    if key not in _prog_cache:
        _prog_cache[key] = build_program(T, ct, BLOC, phases)
    nc = _prog_cache[key]
    in_maps = _host_inputs(x, transitions, seq_lens, lanes)
    res = run_bass_kernel_spmd(nc, in_maps, list(range(NCORES)), trace=TRACE)
    LAST_RESULT = res
    out = np.empty((B, T, C), np.float32)
    for c in range(NCORES):
        out[lanes[c]] = res.results[c]["out"].reshape(BLOC, T, C)
    return out
